# revision 18
# baseline (speedup 1.0000x reference)
"""Trainium2 Bass kernel for a dense transformer block (attention + LoRA +
MLP + proj), data-parallel over batch across 8 NeuronCores.

Contract: kernel(**inputs) takes the FULL unsharded inputs (numpy arrays,
keys as in reference.setup_inputs()) and returns the FULL [8, 512, 1024]
fp32 output.

Design (per core, one batch element):
  - Everything flows channel-major ("transposed"): activations are [C, S]
    tiles with channels on SBUF partitions.  All weights are then used in
    their natural [C_in, C_out] layout (as stationary lhsT slices for
    channel-major outputs, as moving rhs for the token-major v); the only
    transposes in the whole pipeline happen on the host (x -> x.T in,
    out.T -> out).
  - Attention runs keys-on-partitions (attnT = K q^T per head).  The key
    mask is folded into v: masked key ROWS of the token-major v (and of
    its appended ones-columns) are zeroed, which is mathematically
    identical to masking the softmax numerator and denominator.  The
    softmax exp is then a single bias-free ACT op per head, with the
    1/sqrt(hd) scale folded into its scale argument, and the denominator
    comes free as a ones-column appended to v in the PV matmul (M=65).
    Normalization happens per head-pair (overlapped with later heads) via
    a tiny K=2 broadcast matmul of the f32r reciprocals.
  - GEMMs run in bf16 (measured ~2x faster than fp32r per matmul); PSUM
    accumulation is fp32.
"""

import numpy as np

B, S, C = 8, 512, 1024
H, HD, R, HID = 16, 64, 32, 1024
NC3 = 3 * C
NCORES = 8
KC = C // 128          # 8 contraction chunks
MQK = 2 * C // 128     # 16 q+k channel-major output chunks
VSTRIDE = HD + 1       # v columns per head incl. ones column

_cache = {}


def _get_nc():
    if "nc" in _cache:
        return _cache["nc"]

    from contextlib import ExitStack
    import concourse.tile as tile
    from concourse import bacc, mybir

    f32 = mybir.dt.float32
    f32r = mybir.dt.float32r
    bf16 = mybir.dt.bfloat16
    AF = mybir.ActivationFunctionType
    ALU = mybir.AluOpType

    nc = bacc.Bacc("TRN2", target_bir_lowering=False, debug=False)

    def din(name, shape, dt=bf16):
        return nc.dram_tensor(name, list(shape), dt, kind="ExternalInput")

    xT_d = din("xT", (C, S))
    mask01_d = din("mask01", (128, 4), f32)
    sel2_d = din("sel2", (2, 128), f32r)
    qkv_w_d = din("qkv_w", (C, NC3))
    qkv_la_d = din("qkv_la", (C, R))
    qkv_lb_d = din("qkv_lb", (R, NC3))
    proj_w_d = din("proj_w", (C, C))
    proj_b_d = din("proj_b", (C,), f32)
    proj_la_d = din("proj_la", (C, R))
    proj_lb_d = din("proj_lb", (R, C))
    fc1_w_d = din("fc1_w", (C, HID))
    fc1_b_d = din("fc1_b", (HID,), f32)
    fc1_la_d = din("fc1_la", (C, R))
    fc1_lb_d = din("fc1_lb", (R, HID))
    fc2_w_d = din("fc2_w", (HID, C))
    fc2_b_d = din("fc2_b", (C,), f32)
    fc2_la_d = din("fc2_la", (HID, R))
    fc2_lb_d = din("fc2_lb", (R, C))
    outT_d = nc.dram_tensor("outT", [C, S], f32, kind="ExternalOutput")

    with tile.TileContext(nc) as tc, ExitStack() as ctx:
        resident = ctx.enter_context(tc.tile_pool(name="resident", bufs=1))
        wpool = ctx.enter_context(tc.tile_pool(name="wstream", bufs=6))
        psum = ctx.enter_context(tc.tile_pool(name="psum", bufs=2, space="PSUM"))
        expp = ctx.enter_context(tc.tile_pool(name="expp", bufs=2))
        tmpp = ctx.enter_context(tc.tile_pool(name="tmpp", bufs=2))
        outp = ctx.enter_context(tc.tile_pool(name="outp", bufs=2))
        dram = ctx.enter_context(tc.tile_pool(name="dram", bufs=1, space="DRAM"))

        def big_psum(name):
            # [128, 4, S] fp32 = 4 PSUM banks; the only psum tag (2 bufs = all
            # 8 banks).
            return psum.tile([128, 4, S], f32, name=name, tag="big")

        # ---- resident loads -------------------------------------------------
        xT = resident.tile([128, KC, S], bf16, name="xT", tag="xT")
        nc.sync.dma_start(xT[:], xT_d[:].rearrange("(c p) s -> p c s", p=128))
        mask01 = resident.tile([128, 4], f32, name="mask01", tag="mask01")
        nc.sync.dma_start(mask01[:], mask01_d[:])
        sel2 = resident.tile([2, 128], f32r, name="sel2", tag="sel2")
        nc.sync.dma_start(sel2[:], sel2_d[:])

        la = {}
        lb = {}
        for nm, la_d, lb_d, ncols in (
            ("qkv", qkv_la_d, qkv_lb_d, NC3),
            ("fc1", fc1_la_d, fc1_lb_d, HID),
            ("fc2", fc2_la_d, fc2_lb_d, C),
            ("proj", proj_la_d, proj_lb_d, C),
        ):
            la[nm] = resident.tile(
                [128, KC, R], bf16, name=f"la_{nm}", tag=f"la_{nm}"
            )
            nc.sync.dma_start(
                la[nm][:], la_d[:].rearrange("(c p) r -> p c r", p=128)
            )
            lb[nm] = resident.tile(
                [R, ncols], bf16, name=f"lb_{nm}", tag=f"lb_{nm}"
            )
            nc.sync.dma_start(lb[nm][:], lb_d[:])

        biases = {}
        for nm, b_d in (("fc1", fc1_b_d), ("fc2", fc2_b_d), ("proj", proj_b_d)):
            biases[nm] = resident.tile(
                [128, KC], f32, name=f"b_{nm}", tag=f"b_{nm}"
            )
            nc.sync.dma_start(
                biases[nm][:], b_d[:].rearrange("(m p) -> p m", p=128)
            )

        qkv_w_r = qkv_w_d[:].rearrange("(k p) n -> k p n", p=128)
        fc1_w_r = fc1_w_d[:].rearrange("(k p) n -> k p n", p=128)
        fc2_w_r = fc2_w_d[:].rearrange("(k p) n -> k p n", p=128)
        proj_w_r = proj_w_d[:].rearrange("(k p) n -> k p n", p=128)

        def lora_step(nm, pt, act, kc):
            nc.tensor.matmul(
                pt[0:R, 0, :], la[nm][:, kc, :], act[:, kc, :],
                start=(kc == 0), stop=(kc == KC - 1),
            )

        def lora_end(nm, pt):
            t = resident.tile([R, S], bf16, name=f"tT_{nm}", tag=f"tT_{nm}")
            nc.any.tensor_copy(t[:], pt[0:R, 0, :])
            return t

        # ---- qkv GEMM -------------------------------------------------------
        # q,k channel-major: qkT[:, m, :], m in [0,16) covers channels [0,2C)
        qkT = resident.tile([128, MQK, S], bf16, name="qkT", tag="qkT")
        pt_qkv = big_psum("pt_qkv")
        tT_qkv = None
        for g in range(4):            # groups of 4 output chunks
            pg = big_psum(f"pqk{g}")
            for kc in range(KC):
                wt = wpool.tile([128, 512], bf16, tag="w")
                nc.sync.dma_start(
                    wt[:], qkv_w_r[kc, :, g * 512:(g + 1) * 512]
                )
                for i in range(4):
                    nc.tensor.matmul(
                        pg[:, i, :], wt[:, i * 128:(i + 1) * 128],
                        xT[:, kc, :], start=(kc == 0), stop=False,
                    )
                if g == 0:
                    lora_step("qkv", pt_qkv, xT, kc)
            if g == 0:
                tT_qkv = lora_end("qkv", pt_qkv)
            for i in range(4):
                m = g * 4 + i
                nc.tensor.matmul(
                    pg[:, i, :], lb["qkv"][:, m * 128:(m + 1) * 128],
                    tT_qkv[:], start=False, stop=True,
                )
            nc.any.tensor_copy(qkT[:, g * 4:(g + 1) * 4, :], pg[:])

        # v token-major with interleaved ones columns: v[:, c, h*65:+64];
        # masked key rows (incl. their ones entries) are zeroed -> the mask
        # needs no separate handling anywhere else.
        v = resident.tile([128, 4, H * VSTRIDE], bf16, name="vtok", tag="vtok")
        for h in range(H):
            nc.vector.memset(
                v[:, :, h * VSTRIDE + HD:h * VSTRIDE + HD + 1], 1.0
            )
        for c in range(4):
            ones_cols = v[:, c, :].rearrange("p (h z) -> p h z", z=VSTRIDE)[
                :, :, HD:HD + 1
            ]
            nc.vector.tensor_scalar_mul(ones_cols, ones_cols, mask01[:, c:c + 1])
        for n in range(2):
            pg = big_psum(f"pv{n}")
            for kc in range(KC):
                wt = wpool.tile([128, 512], bf16, tag="w")
                nc.sync.dma_start(
                    wt[:], qkv_w_r[kc, :, 2 * C + n * 512:2 * C + (n + 1) * 512]
                )
                for c in range(4):
                    nc.tensor.matmul(
                        pg[:, c, :], xT[:, kc, c * 128:(c + 1) * 128],
                        wt[:], start=(kc == 0), stop=False,
                    )
            for c in range(4):
                nc.tensor.matmul(
                    pg[:, c, :], tT_qkv[:, c * 128:(c + 1) * 128],
                    lb["qkv"][:, 2 * C + n * 512:2 * C + (n + 1) * 512],
                    start=False, stop=True,
                )
                # copy 8 heads' columns into 65-strided slots, zeroing masked
                # key rows on the way
                dst = v[:, c, n * 8 * VSTRIDE:(n + 1) * 8 * VSTRIDE].rearrange(
                    "p (h z) -> p h z", z=VSTRIDE
                )[:, :, 0:HD]
                src = pg[:, c, :].rearrange("p (h z) -> p h z", z=HD)
                nc.vector.tensor_scalar_mul(dst, src, mask01[:, c:c + 1])

        # ---- attention ------------------------------------------------------
        # xou: unnormalized attention output, channel-major [128, KC, S]
        xou = resident.tile([128, KC, S], bf16, name="xou", tag="xou")
        den_dr = dram.tile([128, H, 4], f32r, name="den_dr", tag="den_dr")
        recip_dr = dram.tile([128, H, 4], f32r, name="recip_dr", tag="recip_dr")
        den128 = resident.tile([128, H, 4], f32r, name="den128", tag="den128")
        recip128 = resident.tile([128, H, 4], f32r, name="recip128",
                                 tag="recip128")
        recip2 = resident.tile([2, KC, S], f32r, name="recip2", tag="recip2")

        def pv_head(ph, ppa, pexp, c):
            # PV accumulates into the previous head's pa bank 0 (free after
            # its exp read); row 64 is the softmax denominator via v's ones
            # column.
            nc.tensor.matmul(
                ppa[0:VSTRIDE, 0, :],
                v[:, c, ph * VSTRIDE:(ph + 1) * VSTRIDE],
                pexp[:, c, :],
                start=(c == 0), stop=(c == 3),
            )

        def finish_head(ph, ppa):
            pj, phalf = ph // 2, ph % 2
            tmd = tmpp.tile([128, S], f32r, name="tmd", tag="tmpd")
            nc.vector.tensor_copy(tmd[HD:HD + 1, :], ppa[HD:HD + 1, 0, :])
            nc.sync.dma_start(den_dr[:, ph, :], tmd[HD:HD + 1, :])
            if phalf == 0:
                nc.vector.tensor_copy(xou[0:64, pj, :], ppa[0:HD, 0, :])
            else:
                tmb = tmpp.tile([128, S], bf16, name="tmb", tag="tmpb")
                nc.vector.tensor_copy(tmb[0:HD, :], ppa[0:HD, 0, :])
                nc.sync.dma_start(xou[64:128, pj, :], tmb[0:HD, :])

        prev = None
        for h in range(H):
            j, half = h // 2, h % 2
            p0 = 64 * half
            pa = big_psum("pa")
            # interleave this head's QK with the previous head's PV 1:1 so
            # consecutive PE matmuls never accumulate into the same PSUM bank
            for c in range(4):
                nc.tensor.matmul(
                    pa[:, c, :],
                    qkT[p0:p0 + 64, 8 + j, c * 128:(c + 1) * 128],
                    qkT[p0:p0 + 64, j, :],
                )
                if prev is not None:
                    pv_head(prev[0], prev[1], prev[2], c)
            if prev is not None:
                finish_head(prev[0], prev[1])
            exp_t = expp.tile([128, 4, S], bf16, name="exp_t", tag="exp")
            # exp(attn / sqrt(hd)); masking already folded into v
            nc.scalar.activation(exp_t[:], pa[:], AF.Exp, scale=0.125)
            prev = (h, pa, exp_t)
        for c in range(4):
            pv_head(prev[0], prev[1], prev[2], c)
        finish_head(prev[0], prev[1])

        # normalize: reciprocal reshaped to all 128 partitions (DVE recip cost
        # scales with per-partition free size), redistributed via DRAM, then
        # per chunk a K=2 broadcast matmul + scale; chunk j unblocks fc1's
        # kc=j work via subtile deps.
        nc.sync.dma_start(den128[:], den_dr[:])
        with nc.allow_low_precision(reason="f32r keeps fp32 bits"):
            nc.vector.reciprocal(recip128[:], den128[:])
        nc.sync.dma_start(recip_dr[:], recip128[:])
        recip_dr_t = recip_dr[:].rearrange("p (j t) c -> p j t c", t=2).rearrange(
            "p j t c -> t j p c"
        )
        for t in range(2):
            nc.sync.dma_start(
                recip2[t:t + 1, :, :].rearrange("o j (p c) -> o j p c", c=4),
                recip_dr_t[t],
            )
        for j in range(KC):
            pn = big_psum(f"pn{j}")
            nc.tensor.matmul(pn[:, 0, :], sel2[:], recip2[:, j, :])
            nc.vector.tensor_mul(xou[:, j, :], xou[:, j, :], pn[:, 0, :])
        xoT = xou  # normalized in place

        # ---- MLP fc1 + gelu -------------------------------------------------
        pt_fc1 = big_psum("pt_fc1")
        tT_fc1 = None
        gT = resident.tile([128, KC, S], bf16, name="gT", tag="gT")
        for g in range(2):
            pg = big_psum(f"pf{g}")
            for kc in range(KC):
                wt = wpool.tile([128, 512], bf16, tag="w")
                nc.sync.dma_start(wt[:], fc1_w_r[kc, :, g * 512:(g + 1) * 512])
                for i in range(4):
                    nc.tensor.matmul(
                        pg[:, i, :], wt[:, i * 128:(i + 1) * 128],
                        xoT[:, kc, :], start=(kc == 0), stop=False,
                    )
                if g == 0:
                    lora_step("fc1", pt_fc1, xoT, kc)
            if g == 0:
                tT_fc1 = lora_end("fc1", pt_fc1)
            for i in range(4):
                m = g * 4 + i
                nc.tensor.matmul(
                    pg[:, i, :], lb["fc1"][:, m * 128:(m + 1) * 128],
                    tT_fc1[:], start=False, stop=True,
                )
                nc.scalar.activation(
                    gT[:, m, :], pg[:, i, :], AF.Gelu,
                    bias=biases["fc1"][:, m:m + 1],
                )

        # ---- MLP fc2 + residual --------------------------------------------
        pt_fc2 = big_psum("pt_fc2")
        tT_fc2 = None
        xo2T = resident.tile([128, KC, S], bf16, name="xo2T", tag="xo2T")
        for g in range(2):
            pg = big_psum(f"pg{g}")
            for kc in range(KC):
                wt = wpool.tile([128, 512], bf16, tag="w")
                nc.sync.dma_start(wt[:], fc2_w_r[kc, :, g * 512:(g + 1) * 512])
                for i in range(4):
                    nc.tensor.matmul(
                        pg[:, i, :], wt[:, i * 128:(i + 1) * 128],
                        gT[:, kc, :], start=(kc == 0), stop=False,
                    )
                if g == 0:
                    lora_step("fc2", pt_fc2, gT, kc)
            if g == 0:
                tT_fc2 = lora_end("fc2", pt_fc2)
            for i in range(4):
                m = g * 4 + i
                nc.tensor.matmul(
                    pg[:, i, :], lb["fc2"][:, m * 128:(m + 1) * 128],
                    tT_fc2[:], start=False, stop=True,
                )
                # xo2 = (fc2_psum + bias) + xo  (residual)
                nc.vector.scalar_tensor_tensor(
                    xo2T[:, m, :], pg[:, i, :], biases["fc2"][:, m:m + 1],
                    xoT[:, m, :], op0=ALU.add, op1=ALU.add,
                )

        # ---- proj -----------------------------------------------------------
        pt_proj = big_psum("pt_proj")
        tT_proj = None
        outT_r = outT_d[:].rearrange("(m p) s -> p m s", p=128)
        for g in range(2):
            pg = big_psum(f"pp{g}")
            for kc in range(KC):
                wt = wpool.tile([128, 512], bf16, tag="w")
                nc.sync.dma_start(wt[:], proj_w_r[kc, :, g * 512:(g + 1) * 512])
                for i in range(4):
                    nc.tensor.matmul(
                        pg[:, i, :], wt[:, i * 128:(i + 1) * 128],
                        xo2T[:, kc, :], start=(kc == 0), stop=False,
                    )
                if g == 0:
                    lora_step("proj", pt_proj, xo2T, kc)
            if g == 0:
                tT_proj = lora_end("proj", pt_proj)
            ot = outp.tile([128, 4, S], f32, name="ot", tag="out")
            for i in range(4):
                m = g * 4 + i
                nc.tensor.matmul(
                    pg[:, i, :], lb["proj"][:, m * 128:(m + 1) * 128],
                    tT_proj[:], start=False, stop=True,
                )
                nc.scalar.activation(
                    ot[:, i, :], pg[:, i, :], AF.Identity,
                    bias=biases["proj"][:, m:m + 1],
                )
            nc.sync.dma_start(outT_r[:, g * 4:(g + 1) * 4, :], ot[:])

    nc.compile()
    _cache["nc"] = nc
    return nc


def _bf16(a):
    import ml_dtypes

    return np.asarray(a, dtype=np.float32).astype(ml_dtypes.bfloat16)


def _make_in_maps(inputs):
    x = np.asarray(inputs["x"], dtype=np.float32)
    mask = np.asarray(inputs["mask"])
    sel2 = np.zeros((2, 128), dtype=np.float32)
    sel2[0, 0:64] = 1.0
    sel2[1, 64:128] = 1.0
    shared = {"sel2": sel2}
    for k in (
        "qkv_w", "qkv_la", "qkv_lb", "proj_w", "proj_la", "proj_lb",
        "fc1_w", "fc1_la", "fc1_lb", "fc2_w", "fc2_la", "fc2_lb",
    ):
        shared[k] = np.ascontiguousarray(_bf16(inputs[k]))
    for k in ("proj_b", "fc1_b", "fc2_b"):
        shared[k] = np.ascontiguousarray(inputs[k], dtype=np.float32)
    in_maps = []
    for b in range(NCORES):
        m01 = mask[b, :S].astype(np.float32)          # 1.0 keep / 0.0 drop
        in_maps.append(
            dict(
                shared,
                xT=np.ascontiguousarray(_bf16(x[b].T)),
                mask01=np.ascontiguousarray(m01.reshape(4, 128).T),
            )
        )
    return in_maps


def _run(inputs, trace=False):
    from concourse.bass_utils import run_bass_kernel_spmd

    nc = _get_nc()
    in_maps = _make_in_maps(inputs)
    res = run_bass_kernel_spmd(nc, in_maps, list(range(NCORES)), trace=trace)
    out = np.stack(
        [np.ascontiguousarray(res.results[b]["outT"].T) for b in range(NCORES)]
    )
    return out, res


def kernel(**inputs):
    out, _ = _run(inputs, trace=False)
    return out


# revision 19
# speedup vs baseline: 1.3398x; 1.3398x over previous
"""Trainium2 Bass kernel for a dense transformer block (attention + LoRA +
MLP + proj), data-parallel over batch across 8 NeuronCores.

Contract: kernel(**inputs) takes the FULL unsharded inputs (numpy arrays,
keys as in reference.setup_inputs()) and returns the FULL [8, 512, 1024]
fp32 output.

Design (per core, one batch element):
  - Everything flows channel-major ("transposed"): activations are [C, S]
    tiles with channels on SBUF partitions.  All weights are used in their
    natural [C_in, C_out] layout; the only input/output transposes happen
    on the host.
  - Attention runs keys-on-partitions (attnT = K q^T per head).  The key
    mask is folded into v (masked key rows of token-major v and of its
    appended ones-columns are zeroed), so softmax exp is a bias-free ACT
    op with the 1/sqrt(hd) scale folded in, and the denominator comes free
    as a ones-column in the PV matmul (M=65).  Heads are software-
    pipelined: head h's QK matmuls interleave 1:1 with head h-1's PV
    matmuls (the PE executes its stream in order, so PV - which waits on
    exp - must not block the next head's QK; the interleave also avoids
    back-to-back accumulation into one PSUM bank, which halves matmul
    rate).
  - PSUM: 2-bank "qk2" tiles (x3) released right after exp, 1-bank "pv"
    tiles (x2) that also serve the LoRA-tT and normalization matmuls.
  - Softmax normalization: per-head denominators are scattered to a
    [128, H, 4] layout so the DVE reciprocal runs on all 128 partitions
    (its cost is per-partition-serial), then PE transposes move the
    reciprocals to a [16, 512] queries-on-free layout, and a K=16
    selection matmul broadcasts them per chunk.
  - GEMMs run in bf16 (measured ~2x faster than fp32r); PSUM accumulation
    is fp32; the reciprocal path stays f32r (= fp32 bits).
"""

import numpy as np

B, S, C = 8, 512, 1024
H, HD, R, HID = 16, 64, 32, 1024
NC3 = 3 * C
NCORES = 8
KC = C // 128          # 8 contraction chunks
MQK = 2 * C // 128     # 16 q+k channel-major output chunks
VSTRIDE = HD + 1       # v columns per head incl. ones column

_cache = {}


def _get_nc():
    if "nc" in _cache:
        return _cache["nc"]

    from contextlib import ExitStack
    import concourse.tile as tile
    from concourse import bacc, mybir

    f32 = mybir.dt.float32
    f32r = mybir.dt.float32r
    bf16 = mybir.dt.bfloat16
    AF = mybir.ActivationFunctionType
    ALU = mybir.AluOpType

    nc = bacc.Bacc("TRN2", target_bir_lowering=False, debug=False)

    def din(name, shape, dt=bf16):
        return nc.dram_tensor(name, list(shape), dt, kind="ExternalInput")

    xT_d = din("xT", (C, S))
    mask01_d = din("mask01", (128, 4), f32)
    sel16_d = din("sel16", (H, C), f32r)
    ident_d = din("ident", (128, 128), f32r)
    qkv_w_d = din("qkv_w", (C, NC3))
    qkv_la_d = din("qkv_la", (C, R))
    qkv_lb_d = din("qkv_lb", (R, NC3))
    proj_w_d = din("proj_w", (C, C))
    proj_b_d = din("proj_b", (C,), f32)
    proj_la_d = din("proj_la", (C, R))
    proj_lb_d = din("proj_lb", (R, C))
    fc1_w_d = din("fc1_w", (C, HID))
    fc1_b_d = din("fc1_b", (HID,), f32)
    fc1_la_d = din("fc1_la", (C, R))
    fc1_lb_d = din("fc1_lb", (R, HID))
    fc2_w_d = din("fc2_w", (HID, C))
    fc2_b_d = din("fc2_b", (C,), f32)
    fc2_la_d = din("fc2_la", (HID, R))
    fc2_lb_d = din("fc2_lb", (R, C))
    outT_d = nc.dram_tensor("outT", [C, S], f32, kind="ExternalOutput")

    with tile.TileContext(nc) as tc, ExitStack() as ctx:
        resident = ctx.enter_context(tc.tile_pool(name="resident", bufs=1))
        wpool = ctx.enter_context(tc.tile_pool(name="wstream", bufs=6))
        psum = ctx.enter_context(tc.tile_pool(name="psum", bufs=3, space="PSUM"))
        psum1 = ctx.enter_context(
            tc.tile_pool(name="psum1", bufs=2, space="PSUM")
        )
        expp = ctx.enter_context(tc.tile_pool(name="expp", bufs=2))
        tmpp = ctx.enter_context(tc.tile_pool(name="tmpp", bufs=2))
        outp = ctx.enter_context(tc.tile_pool(name="outp", bufs=2))

        def qk2_psum(name, dt=f32):
            # 2 PSUM banks; 3 bufs -> 6 banks
            return psum.tile([128, 2, S], dt, name=name, tag="qk2")

        def pv_psum(name, dt=f32):
            # 1 PSUM bank; 2 bufs -> 2 banks
            return psum1.tile([128, S], dt, name=name, tag="pv")

        # ---- resident loads -------------------------------------------------
        xT = resident.tile([128, KC, S], bf16, name="xT", tag="xT")
        nc.sync.dma_start(xT[:], xT_d[:].rearrange("(c p) s -> p c s", p=128))
        mask01 = resident.tile([128, 4], f32, name="mask01", tag="mask01")
        nc.sync.dma_start(mask01[:], mask01_d[:])
        sel16 = resident.tile([H, C], f32r, name="sel16", tag="sel16")
        nc.sync.dma_start(sel16[:], sel16_d[:])
        ident = resident.tile([128, 128], f32r, name="ident", tag="ident")
        nc.sync.dma_start(ident[:], ident_d[:])

        la = {}
        lb = {}
        for nm, la_d, lb_d, ncols in (
            ("qkv", qkv_la_d, qkv_lb_d, NC3),
            ("fc1", fc1_la_d, fc1_lb_d, HID),
            ("fc2", fc2_la_d, fc2_lb_d, C),
            ("proj", proj_la_d, proj_lb_d, C),
        ):
            la[nm] = resident.tile(
                [128, KC, R], bf16, name=f"la_{nm}", tag=f"la_{nm}"
            )
            nc.sync.dma_start(
                la[nm][:], la_d[:].rearrange("(c p) r -> p c r", p=128)
            )
            lb[nm] = resident.tile(
                [R, ncols], bf16, name=f"lb_{nm}", tag=f"lb_{nm}"
            )
            nc.sync.dma_start(lb[nm][:], lb_d[:])

        biases = {}
        for nm, b_d in (("fc1", fc1_b_d), ("fc2", fc2_b_d), ("proj", proj_b_d)):
            biases[nm] = resident.tile(
                [128, KC], f32, name=f"b_{nm}", tag=f"b_{nm}"
            )
            nc.sync.dma_start(
                biases[nm][:], b_d[:].rearrange("(m p) -> p m", p=128)
            )

        qkv_w_r = qkv_w_d[:].rearrange("(k p) n -> k p n", p=128)
        fc1_w_r = fc1_w_d[:].rearrange("(k p) n -> k p n", p=128)
        fc2_w_r = fc2_w_d[:].rearrange("(k p) n -> k p n", p=128)
        proj_w_r = proj_w_d[:].rearrange("(k p) n -> k p n", p=128)

        def lora_step(nm, pt, act, kc):
            nc.tensor.matmul(
                pt[0:R, :], la[nm][:, kc, :], act[:, kc, :],
                start=(kc == 0), stop=(kc == KC - 1),
            )

        def lora_end(nm, pt):
            t = resident.tile([R, S], bf16, name=f"tT_{nm}", tag=f"tT_{nm}")
            nc.any.tensor_copy(t[:], pt[0:R, :])
            return t

        def mlp_gemm(nm, w_r, act, epilogue):
            """Generic 1024->1024 GEMM with LoRA; epilogue(m, psum_ap)."""
            pt = pv_psum(f"pt_{nm}")
            tT = None
            for g in range(2):
                pga = qk2_psum(f"p{nm}{g}a")
                pgb = qk2_psum(f"p{nm}{g}b")
                halves = (pga, pgb)
                for kc in range(KC):
                    wt = wpool.tile([128, 512], bf16, tag="w")
                    nc.sync.dma_start(
                        wt[:], w_r[kc, :, g * 512:(g + 1) * 512]
                    )
                    for i in range(4):
                        nc.tensor.matmul(
                            halves[i // 2][:, i % 2, :],
                            wt[:, i * 128:(i + 1) * 128],
                            act[:, kc, :], start=(kc == 0), stop=False,
                        )
                    if g == 0:
                        lora_step(nm, pt, act, kc)
                if g == 0:
                    tT = lora_end(nm, pt)
                for i in range(4):
                    m = g * 4 + i
                    pm = halves[i // 2][:, i % 2, :]
                    nc.tensor.matmul(
                        pm, lb[nm][:, m * 128:(m + 1) * 128],
                        tT[:], start=False, stop=True,
                    )
                    epilogue(m, pm)

        # ---- qkv GEMM -------------------------------------------------------
        # q,k channel-major: qkT[:, m, :], m in [0,16) covers channels [0,2C)
        qkT = resident.tile([128, MQK, S], bf16, name="qkT", tag="qkT")
        pt_qkv = pv_psum("pt_qkv")
        tT_qkv = None
        for g in range(4):            # groups of 4 output chunks
            pga = qk2_psum(f"pqk{g}a")
            pgb = qk2_psum(f"pqk{g}b")
            halves = (pga, pgb)
            for kc in range(KC):
                wt = wpool.tile([128, 512], bf16, tag="w")
                nc.sync.dma_start(
                    wt[:], qkv_w_r[kc, :, g * 512:(g + 1) * 512]
                )
                for i in range(4):
                    nc.tensor.matmul(
                        halves[i // 2][:, i % 2, :],
                        wt[:, i * 128:(i + 1) * 128],
                        xT[:, kc, :], start=(kc == 0), stop=False,
                    )
                if g == 0:
                    lora_step("qkv", pt_qkv, xT, kc)
            if g == 0:
                tT_qkv = lora_end("qkv", pt_qkv)
            for i in range(4):
                m = g * 4 + i
                nc.tensor.matmul(
                    halves[i // 2][:, i % 2, :],
                    lb["qkv"][:, m * 128:(m + 1) * 128],
                    tT_qkv[:], start=False, stop=True,
                )
            nc.any.tensor_copy(qkT[:, g * 4:g * 4 + 2, :], pga[:])
            nc.any.tensor_copy(qkT[:, g * 4 + 2:g * 4 + 4, :], pgb[:])

        # v token-major with interleaved ones columns: v[:, c, h*65:+64];
        # masked key rows (incl. their ones entries) are zeroed -> the mask
        # needs no separate handling anywhere else.
        v = resident.tile([128, 4, H * VSTRIDE], bf16, name="vtok", tag="vtok")
        for h in range(H):
            nc.vector.memset(
                v[:, :, h * VSTRIDE + HD:h * VSTRIDE + HD + 1], 1.0
            )
        for c in range(4):
            ones_cols = v[:, c, :].rearrange("p (h z) -> p h z", z=VSTRIDE)[
                :, :, HD:HD + 1
            ]
            nc.vector.tensor_scalar_mul(ones_cols, ones_cols, mask01[:, c:c + 1])
        for n in range(2):
            pga = qk2_psum(f"pv{n}a")
            pgb = qk2_psum(f"pv{n}b")
            halves = (pga, pgb)
            for kc in range(KC):
                wt = wpool.tile([128, 512], bf16, tag="w")
                nc.sync.dma_start(
                    wt[:], qkv_w_r[kc, :, 2 * C + n * 512:2 * C + (n + 1) * 512]
                )
                for c in range(4):
                    nc.tensor.matmul(
                        halves[c // 2][:, c % 2, :],
                        xT[:, kc, c * 128:(c + 1) * 128],
                        wt[:], start=(kc == 0), stop=False,
                    )
            for c in range(4):
                pm = halves[c // 2][:, c % 2, :]
                nc.tensor.matmul(
                    pm, tT_qkv[:, c * 128:(c + 1) * 128],
                    lb["qkv"][:, 2 * C + n * 512:2 * C + (n + 1) * 512],
                    start=False, stop=True,
                )
                # copy 8 heads' columns into 65-strided slots, zeroing masked
                # key rows on the way
                dst = v[:, c, n * 8 * VSTRIDE:(n + 1) * 8 * VSTRIDE].rearrange(
                    "p (h z) -> p h z", z=VSTRIDE
                )[:, :, 0:HD]
                src = pm.rearrange("p (h z) -> p h z", z=HD)
                nc.vector.tensor_scalar_mul(dst, src, mask01[:, c:c + 1])

        # ---- attention ------------------------------------------------------
        # xou: unnormalized attention output, channel-major [128, KC, S]
        xou = resident.tile([128, KC, S], bf16, name="xou", tag="xou")
        den128 = resident.tile([128, H, 4], f32r, name="den128", tag="den128")
        recip128 = resident.tile(
            [128, H, 4], f32r, name="recip128", tag="recip128"
        )
        recip16 = resident.tile([H, S], f32r, name="recip16", tag="recip16")

        def finish_head(ph, ppv):
            pj, phalf = ph // 2, ph % 2
            tmd = tmpp.tile([128, S], f32r, name="tmd", tag="tmpd")
            nc.vector.tensor_copy(tmd[HD:HD + 1, :], ppv[HD:HD + 1, :])
            nc.sync.dma_start(den128[:, ph, :], tmd[HD:HD + 1, :])
            with nc.allow_low_precision(reason="f32r keeps fp32 bits"):
                nc.vector.reciprocal(recip128[:, ph, :], den128[:, ph, :])
            if phalf == 0:
                nc.vector.tensor_copy(xou[0:64, pj, :], ppv[0:HD, :])
            else:
                tmb = tmpp.tile([128, S], bf16, name="tmb", tag="tmpb")
                nc.vector.tensor_copy(tmb[0:HD, :], ppv[0:HD, :])
                nc.sync.dma_start(xou[64:128, pj, :], tmb[0:HD, :])

        prev = None
        for h in range(H):
            j, half = h // 2, h % 2
            p0 = 64 * half
            qkA = qk2_psum("qkA")
            qkB = qk2_psum("qkB")
            pvt = pv_psum("pvt")
            exp_t = expp.tile([128, 4, S], bf16, name="exp_t", tag="exp")
            # interleave this head's QK with the previous head's PV 1:1: the
            # PE runs its stream in order, so PV (which waits on exp) must
            # not precede the next head's QK; alternating targets also avoids
            # same-bank accumulation stalls.
            for c in range(4):
                qk_dst = qkA[:, c, :] if c < 2 else qkB[:, c - 2, :]
                nc.tensor.matmul(
                    qk_dst,
                    qkT[p0:p0 + 64, 8 + j, c * 128:(c + 1) * 128],
                    qkT[p0:p0 + 64, j, :],
                )
                if prev is not None:
                    ph, pexp, ppv = prev
                    nc.tensor.matmul(
                        ppv[0:VSTRIDE, :],
                        v[:, c, ph * VSTRIDE:(ph + 1) * VSTRIDE],
                        pexp[:, c, :],
                        start=(c == 0), stop=(c == 3),
                    )
                if c == 1:
                    nc.scalar.activation(
                        exp_t[:, 0:2, :], qkA[:], AF.Exp, scale=0.125
                    )
                elif c == 3:
                    nc.scalar.activation(
                        exp_t[:, 2:4, :], qkB[:], AF.Exp, scale=0.125
                    )
            if prev is not None:
                finish_head(prev[0], prev[2])
            prev = (h, exp_t, pvt)
        ph, pexp, ppv = prev
        for c in range(4):
            nc.tensor.matmul(
                ppv[0:VSTRIDE, :],
                v[:, c, ph * VSTRIDE:(ph + 1) * VSTRIDE],
                pexp[:, c, :],
                start=(c == 0), stop=(c == 3),
            )
        finish_head(ph, ppv)

        # move reciprocals to queries-on-free layout via PE transposes, then
        # broadcast per chunk with a K=16 selection matmul and scale xou.
        for cq in range(4):
            tp = pv_psum(f"tp{cq}", dt=f32r)
            nc.tensor.transpose(
                tp[0:H, 0:128], recip128[:, :, cq], ident[:]
            )
            nc.vector.tensor_copy(
                recip16[:, :].rearrange("h (p c) -> h p c", c=4)[:, :, cq],
                tp[0:H, 0:128],
            )
        for j in range(KC):
            pn = pv_psum(f"pn{j}")
            nc.tensor.matmul(
                pn[:], sel16[:, j * 128:(j + 1) * 128], recip16[:]
            )
            nc.vector.tensor_mul(xou[:, j, :], xou[:, j, :], pn[:])
        xoT = xou  # normalized in place

        # ---- MLP fc1 + gelu -------------------------------------------------
        gT = resident.tile([128, KC, S], bf16, name="gT", tag="gT")

        def fc1_epi(m, pm):
            nc.scalar.activation(
                gT[:, m, :], pm, AF.Gelu, bias=biases["fc1"][:, m:m + 1]
            )

        mlp_gemm("fc1", fc1_w_r, xoT, fc1_epi)

        # ---- MLP fc2 + residual --------------------------------------------
        xo2T = resident.tile([128, KC, S], bf16, name="xo2T", tag="xo2T")

        def fc2_epi(m, pm):
            # xo2 = (fc2_psum + bias) + xo  (residual)
            nc.vector.scalar_tensor_tensor(
                xo2T[:, m, :], pm, biases["fc2"][:, m:m + 1],
                xoT[:, m, :], op0=ALU.add, op1=ALU.add,
            )

        mlp_gemm("fc2", fc2_w_r, gT, fc2_epi)

        # ---- proj -----------------------------------------------------------
        outT_r = outT_d[:].rearrange("(m p) s -> p m s", p=128)
        ots = {}

        def proj_epi(m, pm):
            g, i = m // 4, m % 4
            if i == 0:
                ots[g] = outp.tile([128, 4, S], f32, name=f"ot{g}", tag="out")
            nc.scalar.activation(
                ots[g][:, i, :], pm, AF.Identity,
                bias=biases["proj"][:, m:m + 1],
            )
            if i == 3:
                nc.sync.dma_start(
                    outT_r[:, g * 4:(g + 1) * 4, :], ots[g][:]
                )

        mlp_gemm("proj", proj_w_r, xo2T, proj_epi)

    nc.compile()
    _cache["nc"] = nc
    return nc


def _bf16(a):
    import ml_dtypes

    return np.asarray(a, dtype=np.float32).astype(ml_dtypes.bfloat16)


def _make_in_maps(inputs):
    x = np.asarray(inputs["x"], dtype=np.float32)
    mask = np.asarray(inputs["mask"])
    sel16 = np.zeros((H, C), dtype=np.float32)
    for h in range(H):
        sel16[h, h * HD:(h + 1) * HD] = 1.0
    shared = {"sel16": sel16, "ident": np.eye(128, dtype=np.float32)}
    for k in (
        "qkv_w", "qkv_la", "qkv_lb", "proj_w", "proj_la", "proj_lb",
        "fc1_w", "fc1_la", "fc1_lb", "fc2_w", "fc2_la", "fc2_lb",
    ):
        shared[k] = np.ascontiguousarray(_bf16(inputs[k]))
    for k in ("proj_b", "fc1_b", "fc2_b"):
        shared[k] = np.ascontiguousarray(inputs[k], dtype=np.float32)
    in_maps = []
    for b in range(NCORES):
        m01 = mask[b, :S].astype(np.float32)          # 1.0 keep / 0.0 drop
        in_maps.append(
            dict(
                shared,
                xT=np.ascontiguousarray(_bf16(x[b].T)),
                mask01=np.ascontiguousarray(m01.reshape(4, 128).T),
            )
        )
    return in_maps


def _run(inputs, trace=False):
    from concourse.bass_utils import run_bass_kernel_spmd

    nc = _get_nc()
    in_maps = _make_in_maps(inputs)
    res = run_bass_kernel_spmd(nc, in_maps, list(range(NCORES)), trace=trace)
    out = np.stack(
        [np.ascontiguousarray(res.results[b]["outT"].T) for b in range(NCORES)]
    )
    return out, res


def kernel(**inputs):
    out, _ = _run(inputs, trace=False)
    return out
